# revision 29
# baseline (speedup 1.0000x reference)
"""Bilateral slice apply (HDRNet) Trainium2 Bass kernel.

Problem shapes (hardcoded):
  grid:  [4, 12, 8, 16, 16] f32   (B, (NIN+1)*NOUT, GD, GH, GW)
  guide: [4, 1, 1024, 1024] f32   in [0, 1)
  image: [4, 3, 1024, 1024] f32
  out:   [4, 3, 1024, 1024] f32

Sharding: 8 cores = (batch b = k//2, y-half h = k%2).  Each core computes
out[b, :, 512h:512h+512, :] from its guide/image shard and batch-b grid.

Algorithm (per core):
  Clamped-coordinate trilinear: with gyc/gxc/gzc = clip(coord, 0, G-1),
  tent weights over ALL grid indices reproduce the reference's
  floor+clip corner weighting exactly.
  - y-interp: PE matmul  gy[y_band, (c,d,gw')] = Ay_band.T @ grid_r
    (Ay host tents, grid_r host-relaid [gh, (c,d,gw_padded)])
  - x-interp: stride-0 64-px-segment APs over gy's padded gw' columns
    (xi0'(x) = (x+32)//64) + host wx tent constant tiles. Row tiles are
    1152 wide = 18 uniform segments covering AP-x in [-32, 1120); real
    pixels sit in slots [32, 1056).
  - z: dense sum over d of on-device tent weights wz_d (from guide),
    split across DVE (d 3..7) and GpSimd (d 0..2) with separate
    accumulators to run both engines concurrently.
  - apply: out_o = sum_j C[o,j]*img_j + C[o,3].
"""

import os
import sys
import numpy as np

for _p in ("/opt/trn_rl_repo", "/root/.axon_site/_ro/trn_rl_repo"):
    if _p not in sys.path and os.path.isdir(_p):
        sys.path.insert(0, _p)

from contextlib import ExitStack  # noqa: E402

import concourse.bass as bass  # noqa: E402
import concourse.tile as tile  # noqa: E402
from concourse import bacc, mybir  # noqa: E402
from concourse.bass_utils import run_bass_kernel_spmd  # noqa: E402

F32 = mybir.dt.float32
F32R = mybir.dt.float32r
AF = mybir.ActivationFunctionType
ALU = mybir.AluOpType

B, NIN, NOUT = 4, 3, 3
C = (NIN + 1) * NOUT  # 12
GD, GH, GW = 8, 16, 16
H, W = 1024, 1024
HS = H // 2          # rows per core (y-half)
NBAND = HS // 128    # 4 bands of 128 rows
GWP = GW + 3         # padded gw' columns (1 left, 2 right)
NCOL = C * GD * GWP  # free cols of gy
XT = 1152            # tile alloc width; compute uses PXT slots
PXT = 1088           # 17*64: compute width, AP-x in [-32, 1056)
SLO, SHI = 32, 1056  # tile slots holding real pixels x in [0, 1024)
NSEG = 17            # compute segments; slots [0,1088) cover real px
GP_D = (0, 1, 2)     # depth slabs whose products run on GpSimd
DV_D = (3, 4, 5, 6, 7)  # depth slabs whose products run on DVE

_cached = {}


def _tent_weights(n_pix, n_grid):
    """[n_pix, n_grid] tent weights on clamped continuous coords."""
    scale = n_grid / n_pix
    gc = (np.arange(n_pix) + 0.5) * scale - 0.5
    gc = np.clip(gc, 0.0, n_grid - 1)
    idx = np.arange(n_grid)
    w = np.maximum(1.0 - np.abs(gc[:, None] - idx[None, :]), 0.0)
    return w.astype(np.float32)


def _host_consts():
    # Ay tents for each y-half, transposed: [gh, y_local]
    ay = _tent_weights(H, GH)  # [1024, 16]
    ay_t0 = np.ascontiguousarray(ay[:HS].T)
    ay_t1 = np.ascontiguousarray(ay[HS:].T)

    # wx tents in padded-slot coords: slot s <-> x = s - 32
    gx = (np.arange(W) + 0.5) * (GW / W) - 0.5
    gxc = np.clip(gx, 0.0, GW - 1)
    frac = (gxc - np.floor(gxc)).astype(np.float32)
    wx0 = np.zeros((XT,), np.float32)
    wx1 = np.zeros((XT,), np.float32)
    wx0[SLO:SHI] = 1.0 - frac
    wx1[SLO:SHI] = frac
    wx0c = np.broadcast_to(wx0, (128, XT)).copy()
    wx1c = np.broadcast_to(wx1, (128, XT)).copy()
    return ay_t0, ay_t1, wx0c, wx1c


def _relayout_grid(grid_b):
    """grid_b [12, 8, 16, 16] -> grid_r [16, C*GD*GWP] with padded gw'.

    P[k] = G[clip(k-1, 0, 15)] so that column (x+32)//64 (+1) is the
    correct left (right) x-interp source for every pixel incl. clamps.
    """
    gw_idx = np.clip(np.arange(GWP) - 1, 0, GW - 1)
    gp = grid_b[:, :, :, gw_idx]                     # [12, 8, 16, GWP]
    gr = gp.transpose(2, 0, 1, 3).reshape(GH, C * GD * GWP)
    return np.ascontiguousarray(gr).astype(np.float32)


def _build_nc():
    nc = bacc.Bacc("TRN2", target_bir_lowering=False, debug=False,
                   num_devices=8)

    grid_r = nc.dram_tensor("grid_r", [GH, NCOL], F32, kind="ExternalInput").ap()
    guide_d = nc.dram_tensor("guide", [HS, W], F32, kind="ExternalInput").ap()
    img_d = nc.dram_tensor("img", [NIN * HS, W], F32, kind="ExternalInput").ap()
    ay_d = nc.dram_tensor("ay_t", [GH, HS], F32, kind="ExternalInput").ap()
    wx0_d = nc.dram_tensor("wx0c", [128, XT], F32, kind="ExternalInput").ap()
    wx1_d = nc.dram_tensor("wx1c", [128, XT], F32, kind="ExternalInput").ap()
    dneg_d = nc.dram_tensor("dneg", [128, GD], F32, kind="ExternalInput").ap()
    eye_d = nc.dram_tensor("eye", [128, 128], F32, kind="ExternalInput").ap()
    out_d = nc.dram_tensor("out", [NOUT * HS, W], F32, kind="ExternalOutput").ap()

    with tile.TileContext(nc) as tc, ExitStack() as ctx:
        cpool = ctx.enter_context(tc.tile_pool(name="consts", bufs=1))
        gy_pool = ctx.enter_context(tc.tile_pool(name="gy", bufs=2))
        ps_pool = ctx.enter_context(tc.tile_pool(name="ps", bufs=2, space="PSUM"))
        io_pool = ctx.enter_context(tc.tile_pool(name="io", bufs=1))
        wz_pool = ctx.enter_context(tc.tile_pool(name="wz", bufs=2))
        ab_pool = ctx.enter_context(tc.tile_pool(name="ab", bufs=1))
        acc_pool = ctx.enter_context(tc.tile_pool(name="acc", bufs=1))

        ay_sb = cpool.tile([GH, HS], F32, name="ay_sb")
        nc.sync.dma_start(ay_sb[:], ay_d[:, :])
        grid_sb = cpool.tile([GH, NCOL], F32, name="grid_sb")
        nc.sync.dma_start(grid_sb[:], grid_r[:, :])
        wx0_sb = cpool.tile([128, XT], F32, name="wx0_sb")
        nc.sync.dma_start(wx0_sb[:], wx0_d[:, :])
        wx1_sb = cpool.tile([128, XT], F32, name="wx1_sb")
        nc.sync.dma_start(wx1_sb[:], wx1_d[:, :])
        dneg_sb = cpool.tile([128, GD], F32, name="dneg_sb")
        nc.sync.dma_start(dneg_sb[:], dneg_d[:, :])
        eye_sb = cpool.tile([128, 128], F32, name="eye_sb")
        nc.sync.dma_start(eye_sb[:], eye_d[:, :])
        eye_r = cpool.tile([128, 128], F32R, name="eye_r")
        nc.vector.tensor_copy(eye_r[:], eye_sb[:])

        for band in range(NBAND):
            y0 = band * 128
            # ---- y-interp on PE: gy[128, NCOL] ----
            gy = gy_pool.tile([128, NCOL], F32, name="gy")
            nsplit = 4
            cw = NCOL // nsplit
            for s in range(nsplit):
                gy_ps = ps_pool.tile([128, cw], F32, name=f"gy_ps{s}",
                                     tag=f"gy_ps{s % 2}", bufs=1)
                nc.tensor.matmul(
                    gy_ps[:],
                    ay_sb[:, y0:y0 + 128],
                    grid_sb[:, s * cw:(s + 1) * cw],
                    start=True, stop=True,
                )
                nc.scalar.copy(gy[:, s * cw:(s + 1) * cw], gy_ps[:])

            def gview(cc, d, off):
                base = (cc * GD + d) * GWP + off
                return bass.AP(gy.tensor, gy.offset + base,
                               [[NCOL, 128], [1, NSEG], [0, 64]])

            # ---- guide -> clamped z coordinate ----
            guide_t = io_pool.tile([128, XT], F32, name="guide_t", tag="guide")
            nc.sync.dma_start(guide_t[:, SLO:SHI], guide_d[y0:y0 + 128, :])
            gzc = acc_pool.tile([128, XT], F32, name="gzc", tag="gzc")
            nc.vector.tensor_scalar(gzc[:, :PXT], guide_t[:, :PXT], 8.0, -0.5,
                                    ALU.mult, ALU.add)
            nc.vector.tensor_scalar(gzc[:, :PXT], gzc[:, :PXT], 0.0, float(GD - 1),
                                    ALU.max, ALU.min)

            # ---- z tents premultiplied by wx tents: A_d, B_d ----
            A = []
            for d in range(GD):
                ad = wz_pool.tile([128, XT], F32, name=f"ad{d}", tag="ad")
                nc.scalar.activation(ad[:, :PXT], gzc[:, :PXT], AF.Abs,
                                     bias=dneg_sb[:, d:d + 1], scale=1.0)
                wzd = wz_pool.tile([128, XT], F32, name=f"wz{d}", tag="wzd")
                nc.scalar.activation(wzd[:, :PXT], ad[:, :PXT], AF.Relu,
                                     bias=1.0, scale=-1.0)
                ab_d = ab_pool.tile([128, 2 * PXT], F32, name=f"ab{d}",
                                    tag=f"ab{d}")
                nc.vector.tensor_mul(ab_d[:, :PXT], wzd[:, :PXT],
                                     wx0_sb[:, :PXT])
                nc.gpsimd.tensor_mul(ab_d[:, PXT:2 * PXT], wzd[:, :PXT],
                                     wx1_sb[:, :PXT])
                A.append(ab_d)

            # ---- image tiles ----
            imgt = []
            for j in range(NIN):
                it = io_pool.tile([128, XT], F32, name=f"img{j}", tag=f"img{j}")
                nc.sync.dma_start(it[:, SLO:SHI],
                                  img_d[j * HS + y0:j * HS + y0 + 128, :])
                imgt.append(it)

            # ---- per-channel products on DVE/GpSimd, adds on PE ----
            # Products are written f32r; PE identity-matmuls accumulate all
            # 16 of them per channel into PSUM (3 bank-aligned 384-col
            # slices), freeing DVE/GpSimd from the add chains entirely.
            NS3 = 3
            for o in range(NOUT):
                ot = io_pool.tile([128, XT], F32, name=f"ot{o}", tag=f"ot{o}")
                for j in range(NIN + 1):
                    c = o * 4 + j
                    def pview(t):
                        return bass.AP(t.tensor, t.offset,
                                       [[2 * PXT, 128], [PXT, 2],
                                        [64, NSEG], [1, 64]])

                    def gvpair(cc, d):
                        base = (cc * GD + d) * GWP
                        return bass.AP(gy.tensor, gy.offset + base,
                                       [[NCOL, 128], [1, 2],
                                        [1, NSEG], [0, 64]])

                    prods = []
                    for d in DV_D:
                        tV = acc_pool.tile([128, 2 * PXT], F32R, name="tV",
                                           tag="tV", bufs=3)
                        nc.vector.tensor_mul(pview(tV), pview(A[d]),
                                             gvpair(c, d))
                        prods.append(tV)
                    for d in GP_D:
                        tG = acc_pool.tile([128, 2 * PXT], F32R, name="tG",
                                           tag="tG", bufs=2)
                        nc.gpsimd.tensor_mul(pview(tG), pview(A[d]),
                                             gvpair(c, d))
                        prods.append(tG)
                    acc = ps_pool.tile([128, NS3 * 512], F32, name="cacc",
                                       tag="cacc", bufs=2)
                    slices = [(0, 384), (384, 384), (768, 320)]
                    for i, t in enumerate(prods):
                        for side in range(2):
                            for sl, (t0, tw) in enumerate(slices):
                                nc.tensor.matmul(
                                    acc[:, sl * 512:sl * 512 + tw],
                                    eye_r[:],
                                    t[:, side * PXT + t0:side * PXT + t0 + tw],
                                    start=(i == 0 and side == 0),
                                    stop=(i == len(prods) - 1 and side == 1),
                                )
                    # psum holds [384,384,320] slices at 512-col banks;
                    # fold the first two as one 2x384 view, the tail alone
                    cj2 = bass.AP(acc.tensor, acc.offset,
                                  [[NS3 * 512, 128], [512, 2], [1, 384]])
                    cjt = bass.AP(acc.tensor, acc.offset + 1024,
                                  [[NS3 * 512, 128], [1, 320]])

                    # fold into out_o
                    if j == 0:
                        nc.vector.tensor_mul(ot[:, :768], cj2, imgt[0][:, :768])
                        nc.vector.tensor_mul(ot[:, 768:PXT], cjt,
                                             imgt[0][:, 768:PXT])
                    elif j < NIN:
                        tA = acc_pool.tile([128, XT], F32, name="tA", tag="tA")
                        nc.vector.tensor_mul(tA[:, :768], cj2, imgt[j][:, :768])
                        nc.vector.tensor_mul(tA[:, 768:PXT], cjt,
                                             imgt[j][:, 768:PXT])
                        nc.vector.tensor_add(ot[:, :PXT], ot[:, :PXT],
                                             tA[:, :PXT])
                    else:
                        nc.vector.tensor_add(ot[:, :768], ot[:, :768], cj2)
                        nc.vector.tensor_add(ot[:, 768:PXT], ot[:, 768:PXT],
                                             cjt)
                nc.sync.dma_start(out_d[o * HS + y0:o * HS + y0 + 128, :],
                                  ot[:, SLO:SHI])

    nc.compile()
    return nc


def _get_nc():
    if "nc" not in _cached:
        _cached["nc"] = _build_nc()
    return _cached["nc"]


def kernel(grid, guide, image):
    grid = np.asarray(grid, dtype=np.float32)
    guide = np.asarray(guide, dtype=np.float32)
    image = np.asarray(image, dtype=np.float32)

    nc = _get_nc()
    ay_t0, ay_t1, wx0c, wx1c = _host_consts()
    ay_halves = (ay_t0, ay_t1)

    in_maps = []
    for k in range(8):
        b, h = k // 2, k % 2
        in_maps.append({
            "grid_r": _relayout_grid(grid[b]),
            "guide": np.ascontiguousarray(guide[b, 0, h * HS:(h + 1) * HS, :]),
            "img": np.ascontiguousarray(
                image[b, :, h * HS:(h + 1) * HS, :]).reshape(NIN * HS, W),
            "ay_t": ay_halves[h],
            "wx0c": wx0c,
            "wx1c": wx1c,
            "dneg": np.broadcast_to(
                -np.arange(GD, dtype=np.float32), (128, GD)).copy(),
            "eye": np.eye(128, dtype=np.float32),
        })

    res = run_bass_kernel_spmd(nc, in_maps, core_ids=list(range(8)))

    out = np.empty((B, NOUT, H, W), np.float32)
    for k in range(8):
        b, h = k // 2, k % 2
        out[b, :, h * HS:(h + 1) * HS, :] = \
            res.results[k]["out"].reshape(NOUT, HS, W)
    return out
